# revision 30
# baseline (speedup 1.0000x reference)
"""Trainium2 Bass kernel for CIAttention (RoPE multi-head attention block).

Full computation:
  q/k/v = x @ W{q,k,v}.T  (per-head split), rope(q), rope(k),
  attn = softmax(q k^T / sqrt(hd)), out = (attn @ v) concat -> @ Wo.T

Sharding over 8 NeuronCores: core c handles batch b=c//2 and head-group
g=c%2 (8 of 16 heads). Megatron-style: o_proj produces partial outputs
that the host sums per batch (the tensor-parallel AllReduce done on host).

All matmuls run in bf16 with fp32 PSUM accumulation. Attention math:
scores are computed transposed (S_T[j,i] = k_j . q_i) so the attn@V
contraction needs no on-chip transposes; softmax skips max-subtraction
(|scores| <= ~7 here so exp is safe). The softmax denominator is built by
summing the 16 exp'd chunks with bf16 adds on the vector engine, then one
all-ones matmul sums over partitions and broadcasts the result.

The attention phase alone is ACT(exp)-throughput-bound (per 512-col chunk
the scalar engine needs ~577ns vs ~426ns of PE work), so Q/K projection
matmuls for head h+1 are software-pipelined INTO the attention loop of
head h (2 projection matmuls between each chunk), keeping the PE busy
while ACT drains exps. qt/kt live in a 2-head ring buffer to fit SBUF.
"""

import numpy as np
import ml_dtypes

import concourse.tile as tile
from concourse import bacc, mybir
from concourse.bass_utils import run_bass_kernel_spmd

BF16 = ml_dtypes.bfloat16

D = 2048          # model dim
S = 2048          # sequence length
B = 4             # batch
H_LOC = 8         # heads per core (16 total / 2 groups)
E_LOC = 1024      # local projection dim (8 heads * 128)
HD = 128          # head dim
INV_SQRT_HD = 1.0 / float(np.sqrt(HD))

_CACHE = {}

# tuning knobs (overridable for experiments)
KNOBS = dict(
    pss_bufs=2,    # scores psum pair-tiles (2 banks each) in flight
    psqk_bufs=2,   # q/k projection psum quarter-tiles
    psv_bufs=3,    # v projection psum accumulators (chase the x DMA front)
    po_bufs=2,     # o_proj psum accumulators
    pso_bufs=1,    # attn@v accumulators
    attn_bufs=2,   # per-(h,ic) exp'd score tiles
    out_bf16=True,
    pool_accB=False,  # run the accB chunk-sum chain on GpSimd instead of DVE
)


def _build_nc(**overrides):
    knobs = dict(KNOBS)
    knobs.update(overrides)
    f32 = mybir.dt.float32
    bf16 = mybir.dt.bfloat16
    FT = mybir.ActivationFunctionType

    nc = bacc.Bacc("TRN2", debug=False)

    # Inputs, host-swizzled so every DMA has contiguous >=2KB runs.
    xq_d = nc.dram_tensor("xq", [128, 16, S], bf16, kind="ExternalInput")
    wq_d = nc.dram_tensor("wq", [H_LOC, 128, 16, 128], bf16, kind="ExternalInput")
    wk_d = nc.dram_tensor("wk", [H_LOC, 128, 16, 128], bf16, kind="ExternalInput")
    wv_d = nc.dram_tensor("wv", [128, 16, E_LOC], bf16, kind="ExternalInput")
    wo_d = nc.dram_tensor("wo", [128, 8, D], bf16, kind="ExternalInput")
    cos_d = nc.dram_tensor("cosf", [128, S], bf16, kind="ExternalInput")
    sin_d = nc.dram_tensor("sinf", [128, S], bf16, kind="ExternalInput")
    # Partial output, transposed: outt[e, s]; host adds the two head-group
    # partials per batch and transposes back.
    out_dt = bf16 if knobs["out_bf16"] else f32
    out_d = nc.dram_tensor("outt", [D, S], out_dt, kind="ExternalOutput")

    with tile.TileContext(nc) as tc:
        _emit(tc, nc, f32, bf16, FT,
              xq_d, wq_d, wk_d, wv_d, wo_d, cos_d, sin_d, out_d, knobs)
    nc.compile()
    return nc


def _emit(tc, nc, f32, bf16, FT,
          xq_d, wq_d, wk_d, wv_d, wo_d, cos_d, sin_d, out_d, knobs):
    from contextlib import ExitStack
    with ExitStack() as top:
        consts = top.enter_context(tc.tile_pool(name="consts", bufs=1))
        ones_sb = consts.tile([128, 128], bf16)
        nc.vector.memset(ones_sb[:], 1.0)

        # qt/kt ring: only heads h (in use) and h+1 (being produced) live.
        qk_pool = top.enter_context(tc.tile_pool(name="qk", bufs=2))
        v_pool = top.enter_context(tc.tile_pool(name="v_pool", bufs=1))
        v_sb = v_pool.tile([128, 16, E_LOC], bf16, tag="v")
        aot_pool = top.enter_context(tc.tile_pool(name="aot_pool", bufs=1))
        aot_sb = aot_pool.tile([128, H_LOC, S], bf16, tag="aot")
        cs = top.enter_context(tc.tile_pool(name="cs", bufs=1))
        cos_sb = cs.tile([128, S], bf16, tag="cos")
        sin_sb = cs.tile([128, S], bf16, tag="sin")
        wpool = top.enter_context(tc.tile_pool(name="w1", bufs=2))
        ps1 = top.enter_context(
            tc.tile_pool(name="ps1", bufs=knobs["psqk_bufs"], space="PSUM"))
        rt = top.enter_context(tc.tile_pool(name="ropet", bufs=1))

        # x pool closed manually once QK(7) has consumed it (during the
        # h==6 attention slots) to make room for wo in the h==7 stretch.
        # Both live on the right-side SBUF stack so the early release
        # keeps each side's pool stack properly nested.
        x_cm = tc.tile_pool(name="x_pool", bufs=1, side="right")
        x_pool = x_cm.__enter__()
        xsb = x_pool.tile([128, 16, S], bf16, tag="x")

        # ---- V projection in natural [s, hd] layout ----
        with tc.tile_pool(name="wv_p", bufs=1) as wvp, \
             tc.tile_pool(name="psv", bufs=knobs["psv_bufs"], space="PSUM") as psv:
            wv_sb = wvp.tile([128, 16, E_LOC], bf16)
            # Half-granularity loads split across both HWDGE queues (SP +
            # ACT) so the V matmuls can chase the DMA arrival front: the
            # first s-halves + wv halves enable sc tiles 0..7 completely.
            # Tiny first slices so matmul (sc=0, dc=0) starts ASAP.
            nc.sync.dma_start(out=xsb[:, 0, 0:128],
                              in_=xq_d.ap()[:, 0, 0:128])
            nc.scalar.dma_start(out=wv_sb[:, 0, 0:512],
                                in_=wv_d.ap()[:, 0, 0:512])
            nc.sync.dma_start(out=xsb[:, 0, 128:1024],
                              in_=xq_d.ap()[:, 0, 128:1024])
            nc.scalar.dma_start(out=wv_sb[:, 0, 512:1024],
                                in_=wv_d.ap()[:, 0, 512:1024])
            for dc in range(1, 16):
                nc.sync.dma_start(out=xsb[:, dc, 0:1024],
                                  in_=xq_d.ap()[:, dc, 0:1024])
                nc.scalar.dma_start(out=wv_sb[:, dc, 0:512],
                                    in_=wv_d.ap()[:, dc, 0:512])
                nc.scalar.dma_start(out=wv_sb[:, dc, 512:1024],
                                    in_=wv_d.ap()[:, dc, 512:1024])
            nc.scalar.dma_start(out=cos_sb[:], in_=cos_d.ap())
            nc.scalar.dma_start(out=sin_sb[:], in_=sin_d.ap())
            for dc in range(16):
                nc.sync.dma_start(out=xsb[:, dc, 1024:2048],
                                  in_=xq_d.ap()[:, dc, 1024:2048])
            for sc in range(16):
                ps = psv.tile([128, E_LOC], f32, tag="psv")
                for dc in range(16):
                    for nb in range(2):
                        nsl = slice(nb * 512, (nb + 1) * 512)
                        nc.tensor.matmul(
                            ps[:, nsl], xsb[:, dc, sc * 128:(sc + 1) * 128],
                            wv_sb[:, dc, nsl],
                            start=(dc == 0), stop=(dc == 15))
                nc.scalar.copy(v_sb[:, sc, :], ps[:])

        # RoPE pair lanes sit 16 apart within each 32-partition block (the
        # host permutes wq/wk/cos/sin rows to this layout; scores are
        # invariant under any shared hd permutation), so the half-swap is
        # a single DVE stream_shuffle instead of two ACT partition-copies.
        SWAP16 = list(range(16, 32)) + list(range(0, 16))

        def gen_qk(h, qt_t, kt_t):
            """Q/K projection + RoPE for head h, as a generator yielding
            after every 2 matmuls (64 yields total) so the caller can
            interleave it into the attention slots of head h-1."""
            for w_d, out_t in ((wq_d, qt_t), (wk_d, kt_t)):
                wcol = wpool.tile([128, 16, 128], bf16, tag="wcol")
                nc.sync.dma_start(out=wcol[:], in_=w_d.ap()[h])
                for qtr in range(4):
                    qsl = slice(qtr * 512, (qtr + 1) * 512)
                    ps = ps1.tile([128, 512], f32, tag="psqk")
                    for dc in range(16):
                        nc.tensor.matmul(
                            ps[:], wcol[:, dc, :], xsb[:, dc, qsl],
                            start=(dc == 0), stop=(dc == 15))
                        if dc % 2 == 1:
                            yield
                    # out[q1 lane] = q1*cos - q2*sin
                    # out[q2 lane] = q1*sin + q2*cos
                    # sinf carries [+sin on q1 lanes; -sin on q2 lanes].
                    tmpA = rt.tile([128, 512], f32, tag="tA")
                    tmpB = rt.tile([128, 512], f32, tag="tB")
                    tmpBr = rt.tile([128, 512], f32, tag="tBr")
                    nc.vector.tensor_mul(tmpA[:], ps[:], cos_sb[:, qsl])
                    nc.vector.tensor_mul(tmpB[:], ps[:], sin_sb[:, qsl])
                    nc.vector.stream_shuffle(tmpBr[:], tmpB[:], SWAP16)
                    nc.vector.tensor_add(out_t[:, qsl], tmpA[:], tmpBr[:])

        qt_t = {0: qk_pool.tile([128, S], bf16, tag="qt", name="qt0")}
        kt_t = {0: qk_pool.tile([128, S], bf16, tag="kt", name="kt0")}
        for _ in gen_qk(0, qt_t[0], kt_t[0]):
            pass

        # ---- attention, with QK(h+1) matmuls pipelined into the slots ----
        wo_sb = None
        with tc.tile_pool(name="at", bufs=knobs["attn_bufs"]) as atp, \
             tc.tile_pool(name="acc_p", bufs=1) as accp, \
             tc.tile_pool(name="rc_p", bufs=2) as rcp, \
             tc.tile_pool(name="pss", bufs=knobs["pss_bufs"], space="PSUM") as pssp, \
             tc.tile_pool(name="pso", bufs=knobs["pso_bufs"], space="PSUM") as psop, \
             tc.tile_pool(name="psr", bufs=1, space="PSUM") as psrp:
            for h in range(H_LOC):
                if h + 1 < H_LOC:
                    qt_t[h + 1] = qk_pool.tile([128, S], bf16, tag="qt",
                                               name=f"qt{h + 1}")
                    kt_t[h + 1] = qk_pool.tile([128, S], bf16, tag="kt",
                                               name=f"kt{h + 1}")
                    g = gen_qk(h + 1, qt_t[h + 1], kt_t[h + 1])
                else:
                    # xsb fully consumed by QK(7) during the h==6 slots;
                    # free its 64KB/part and start streaming wo there.
                    x_cm.__exit__(None, None, None)
                    wo_pool = top.enter_context(
                        tc.tile_pool(name="wo_p", bufs=1, side="right"))
                    wo_sb = wo_pool.tile([128, 8, D], bf16)
                    for cc in range(8):
                        nc.sync.dma_start(out=wo_sb[:, cc, :],
                                          in_=wo_d.ap()[:, cc, :])
                    g = None
                hsl = slice(h * 128, (h + 1) * 128)

                def chunk_add(attn, accA, accB, c):
                    acc = accA if c < 8 else accB
                    eng = (nc.gpsimd if (c >= 8 and knobs["pool_accB"])
                           else nc.vector)
                    if c % 8 == 1:
                        eng.tensor_add(
                            acc[:], attn[:, c - 1, :], attn[:, c, :])
                    elif c % 8 > 1:
                        eng.tensor_add(acc[:], acc[:], attn[:, c, :])

                for ic in range(4):
                    isl = slice(ic * 512, (ic + 1) * 512)
                    attn = atp.tile([128, 16, 512], bf16, tag="attn")
                    so = psop.tile([128, 512], f32, tag="pso")
                    accA = accp.tile([128, 512], bf16, tag="accA")
                    accB = accp.tile([128, 512], bf16, tag="accB")
                    # 8 pair-slots: 2 scores matmuls into one 2-bank psum
                    # tile, one [128,1024] exp (amortizes ACT's fixed
                    # overhead), 4 pipelined QK matmuls, then the 2 attn@v
                    # matmuls of the PREVIOUS pair (one pair late so the
                    # exp has a full slot of PE work to hide behind).
                    for p in range(8):
                        ss = pssp.tile([128, 2, 512], f32, tag="pss")
                        # One QK filler between the two scores matmuls so
                        # the pss-bank wait (on exp of pair p-2) has extra
                        # slack before the PE reaches the dependent matmul.
                        jsl = slice(2 * p * 128, (2 * p + 1) * 128)
                        nc.tensor.matmul(
                            ss[:, 0, :], kt_t[h][:, jsl],
                            qt_t[h][:, isl], start=True, stop=True)
                        if g is not None:
                            next(g, None)
                        jsl = slice((2 * p + 1) * 128, (2 * p + 2) * 128)
                        nc.tensor.matmul(
                            ss[:, 1, :], kt_t[h][:, jsl],
                            qt_t[h][:, isl], start=True, stop=True)
                        nc.scalar.activation(
                            attn[:, 2 * p:2 * p + 2, :], ss[:], FT.Exp,
                            scale=INV_SQRT_HD)
                        if g is not None:
                            next(g, None)
                        if p > 0:
                            for c in (2 * p - 2, 2 * p - 1):
                                nc.tensor.matmul(
                                    so[:], v_sb[:, c, hsl], attn[:, c, :],
                                    start=(c == 0), stop=False)
                                chunk_add(attn, accA, accB, c)
                    for c in (14, 15):
                        nc.tensor.matmul(
                            so[:], v_sb[:, c, hsl], attn[:, c, :],
                            start=False, stop=(c == 15))
                        chunk_add(attn, accA, accB, c)
                    csum = rcp.tile([128, 512], bf16, tag="csum")
                    nc.vector.tensor_add(csum[:], accA[:], accB[:])
                    sr = psrp.tile([128, 512], f32, tag="psr")
                    nc.tensor.matmul(sr[:], ones_sb[:], csum[:],
                                     start=True, stop=True)
                    rc = rcp.tile([128, 512], f32, tag="rc")
                    nc.vector.reciprocal_approx_fast(rc[:], sr[:])
                    nc.vector.tensor_mul(aot_sb[:, h, isl], so[:], rc[:])
                if g is not None:
                    for _ in g:
                        pass

        # ---- o_proj partial, output transposed [e, s] ----
        with tc.tile_pool(name="po", bufs=knobs["po_bufs"], space="PSUM") as pop, \
             tc.tile_pool(name="ost", bufs=3) as ostp:
            out_dt = bf16 if knobs["out_bf16"] else f32
            for ec in range(16):
                esl = slice(ec * 128, (ec + 1) * 128)
                for sc4 in range(4):
                    ssl = slice(sc4 * 512, (sc4 + 1) * 512)
                    po = pop.tile([128, 512], f32, tag="po")
                    for cc in range(8):
                        nc.tensor.matmul(
                            po[:], wo_sb[:, cc, esl], aot_sb[:, cc, ssl],
                            start=(cc == 0), stop=(cc == 7))
                    ost = ostp.tile([128, 512], out_dt, tag="ost")
                    nc.vector.tensor_copy(ost[:], po[:])
                    nc.sync.dma_start(out=out_d.ap()[esl, ssl], in_=ost[:])


def get_nc():
    if "nc" not in _CACHE:
        _CACHE["nc"] = _build_nc()
    return _CACHE["nc"]


def _rope_perm():
    """Permutation over the 128 hd lanes: RoPE pair (d, d+64) lands at
    lanes (b*32+l, b*32+16+l) with d = b*16+l — pairs sit 16 apart inside
    each 32-partition block so the kernel's stream_shuffle can swap them.
    perm[p] = original hd dim stored at lane p."""
    perm = np.empty(128, np.int64)
    for b in range(4):
        for l in range(16):
            perm[b * 32 + l] = b * 16 + l           # q1 dims 0..63
            perm[b * 32 + 16 + l] = 64 + b * 16 + l  # q2 dims 64..127
    return perm


def make_in_maps(x, cos, sin, Wq, Wk, Wv, Wo):
    """Host-side shard + swizzle. Returns the 8 per-core input dicts."""
    x = np.asarray(x, np.float32)
    perm = _rope_perm()
    cosT = np.ascontiguousarray(np.asarray(cos, np.float32).T).astype(BF16)
    sinT = np.ascontiguousarray(np.asarray(sin, np.float32).T).astype(BF16)
    cosf = np.concatenate([cosT, cosT], 0)[perm]  # [128, S]
    # +sin on q1 lanes, -sin on q2 lanes: after the 16-lane swap of
    # (ps * sinf), q1 lanes hold -q2*sin and q2 lanes hold +q1*sin.
    sinf = np.concatenate([sinT, -sinT], 0)[perm]
    cosf = np.ascontiguousarray(cosf)
    sinf = np.ascontiguousarray(sinf)

    per_g = []
    for g in range(2):
        wq_loc = np.asarray(Wq, np.float32)[g * E_LOC:(g + 1) * E_LOC].astype(BF16)
        wk_loc = np.asarray(Wk, np.float32)[g * E_LOC:(g + 1) * E_LOC].astype(BF16)
        wv_loc = np.asarray(Wv, np.float32)[g * E_LOC:(g + 1) * E_LOC].astype(BF16)
        wo_loc = np.asarray(Wo, np.float32)[:, g * E_LOC:(g + 1) * E_LOC].astype(BF16)
        # wq_sw[h, p, c, e] = wq_loc[h*128+e, c*128+p], hd lanes permuted
        wq_sw = np.ascontiguousarray(
            wq_loc.reshape(H_LOC, 128, 16, 128)[:, perm].transpose(0, 3, 2, 1))
        wk_sw = np.ascontiguousarray(
            wk_loc.reshape(H_LOC, 128, 16, 128)[:, perm].transpose(0, 3, 2, 1))
        # wv_sw[p, c, e] = wv_loc[e, c*128+p]
        wv_sw = np.ascontiguousarray(
            wv_loc.reshape(E_LOC, 16, 128).transpose(2, 1, 0))
        # wo_sw[p, cc, e] = wo_loc[e, cc*128+p]
        wo_sw = np.ascontiguousarray(
            wo_loc.reshape(D, 8, 128).transpose(2, 1, 0))
        per_g.append((wq_sw, wk_sw, wv_sw, wo_sw))

    per_b = []
    for b in range(B):
        xT = np.ascontiguousarray(x[b].astype(BF16).T)  # [d, s]
        xq_sw = np.ascontiguousarray(xT.reshape(16, 128, S).transpose(1, 0, 2))
        per_b.append(xq_sw)

    in_maps = []
    for c in range(8):
        b, g = divmod(c, 2)
        wq_sw, wk_sw, wv_sw, wo_sw = per_g[g]
        in_maps.append(dict(xq=per_b[b], wq=wq_sw, wk=wk_sw,
                            wv=wv_sw, wo=wo_sw, cosf=cosf, sinf=sinf))
    return in_maps


def assemble_output(results):
    """results: list of 8 dicts with 'outt' [e, s]. Returns [B, S, D] f32."""
    out = np.empty((B, S, D), np.float32)
    for b in range(B):
        acc = (results[2 * b]["outt"].astype(np.float32)
               + results[2 * b + 1]["outt"].astype(np.float32))
        out[b] = acc.T
    return out


def _get_runner():
    """Cached sharded-jit runner (replicates bass2jax.run_bass_via_pjrt's
    shard_map path, with output zero-buffers created on device)."""
    if "runner" in _CACHE:
        return _CACHE["runner"]
    import jax
    import jax.numpy as jnp
    from jax.sharding import Mesh, PartitionSpec, NamedSharding
    from jax.experimental.shard_map import shard_map
    from concourse import bass2jax
    from concourse.bass2jax import _bass_exec_p, partition_id_tensor

    nc = get_nc()
    bass2jax.install_neuronx_cc_hook()
    n_cores = 8
    partition_name = nc.partition_id_tensor.name if nc.partition_id_tensor else None
    in_names, out_names, out_avals, zero_shapes, in_shapes = [], [], [], [], []
    for alloc in nc.m.functions[0].allocations:
        if not isinstance(alloc, mybir.MemoryLocationSet):
            continue
        name = alloc.memorylocations[0].name
        if alloc.kind == "ExternalInput":
            if name != partition_name:
                in_names.append(name)
                in_shapes.append((tuple(alloc.tensor_shape),
                                  mybir.dt.np(alloc.dtype)))
        elif alloc.kind == "ExternalOutput":
            shape = tuple(alloc.tensor_shape)
            dtype = mybir.dt.np(alloc.dtype)
            out_names.append(name)
            out_avals.append(jax.core.ShapedArray(shape, dtype))
            zero_shapes.append((shape, dtype))

    n_params = len(in_names)
    n_outs = len(out_avals)
    all_in_names = list(in_names) + list(out_names)
    if partition_name is not None:
        all_in_names.append(partition_name)

    def _body(*args):
        operands = list(args)
        if partition_name is not None:
            operands.append(partition_id_tensor())
        outs = _bass_exec_p.bind(
            *operands,
            out_avals=tuple(out_avals),
            in_names=tuple(all_in_names),
            out_names=tuple(out_names),
            lowering_input_output_aliases=(),
            sim_require_finite=True,
            sim_require_nnan=True,
            nc=nc,
        )
        return tuple(outs)

    devices = jax.devices()[:n_cores]
    mesh = Mesh(np.asarray(devices), ("core",))
    in_specs = (PartitionSpec("core"),) * (n_params + n_outs)
    out_specs = (PartitionSpec("core"),) * n_outs
    donate = tuple(range(n_params, n_params + n_outs))
    sharding = NamedSharding(mesh, PartitionSpec("core"))

    # AOT-compile with bass_effect suppressed so per-call dispatch takes
    # jax's C++ fast path instead of slow python effects dispatch.
    arg_structs = [
        jax.ShapeDtypeStruct((n_cores * shp[0], *shp[1:]), dt,
                             sharding=sharding)
        for shp, dt in in_shapes + zero_shapes]

    def _compile():
        jitted = jax.jit(
            shard_map(_body, mesh=mesh, in_specs=in_specs,
                      out_specs=out_specs, check_rep=False),
            donate_argnums=donate,
            keep_unused=True,
        )
        return jitted.lower(*arg_structs).compile()

    sharded = bass2jax.fast_dispatch_compile(_compile)
    zero_fn = jax.jit(
        lambda: tuple(
            jnp.zeros((n_cores * shp[0], *shp[1:]), dt)
            for shp, dt in zero_shapes),
        out_shardings=tuple(sharding for _ in zero_shapes),
    )

    # Per-batch pair reduction on device: partial(core 2b) + partial(core
    # 2b+1), transposed back to [s, e] and cast bf16 (one rounding of the
    # final output; halves the slow host<->terminal fetch).
    pair_add = jax.jit(
        lambda a, b: (a.astype(jnp.float32) + b.astype(jnp.float32))
        .T.astype(jnp.bfloat16))

    def run(in_maps):
        # The axon tunnel is slow (~90 MB/s) but device-to-device copies are
        # fast, so upload each unique host array once and replicate on device.
        uploaded = {}  # id(np array) -> {core: device_array}

        def shard_for(arr, c):
            ent = uploaded.setdefault(id(arr), {})
            if c in ent:
                return ent[c]
            if ent:
                src = next(iter(ent.values()))
                a = jax.device_put(src, devices[c])
            else:
                a = jax.device_put(arr, devices[c])
            ent[c] = a
            return a

        args = []
        for name in in_names:
            shards = [shard_for(np.asarray(m[name]), c)
                      for c, m in enumerate(in_maps)]
            a0 = np.asarray(in_maps[0][name])
            gshape = (n_cores * a0.shape[0], *a0.shape[1:])
            args.append(jax.make_array_from_single_device_arrays(
                gshape, sharding, shards))
        args.extend(zero_fn())
        outs = sharded(*args)
        out0 = outs[0]
        summed = []
        for b in range(n_cores // 2):
            s0 = out0.addressable_shards[2 * b].data
            s1 = out0.addressable_shards[2 * b + 1].data
            s1m = jax.device_put(s1, devices[2 * b])
            summed.append(pair_add(s0, s1m))
        for s in summed:
            try:
                s.copy_to_host_async()
            except Exception:
                pass
        return [np.asarray(s) for s in summed]

    _CACHE["runner"] = run
    return run


def kernel(x, cos, sin, Wq, Wk, Wv, Wo):
    in_maps = make_in_maps(x, cos, sin, Wq, Wk, Wv, Wo)
    run = _get_runner()
    partials = run(in_maps)  # 4 arrays [s, e] bf16 (per batch)
    out = np.empty((B, S, D), np.float32)
    for b in range(B):
        out[b] = partials[b]
    return out


if __name__ == "__main__":
    # quick self-build check
    get_nc()
    print("built + compiled OK")
